# revision 23
# baseline (speedup 1.0000x reference)
"""Trainium2 Bass kernel for DecisionTreeModule forward (v4 hybrid).

Levels 0-6: PE matmul of a fixed one-hot [feature->node] matrix against
host-pre-transposed fp16 x gives all 127 node values per sample; DVE compare
+ PE transpose yield sample-major traversal bits; the 7-level walk is narrow
(<=64-wide) selects. Levels 7-11: the baseline's proven structure — resident
fp16 sample-major x tile, 256-wide one-hot selects (fp16 products at 2x),
62-float subtree records fetched by per-column indirect DMA. Output: fp16
softmax rows gathered by leaf; host upcasts to f32 (exact).
"""
import sys
sys.path.insert(0, "/opt/trn_rl_repo")

import numpy as np
import concourse.bacc as bacc
import concourse.bass as bass
import concourse.mybir as mybir
import concourse.tile as tile
from concourse.bass_utils import run_bass_kernel_spmd
from concourse.masks import make_identity

P = 128
INPUT_DIM = 256
N_CLASSES = 100
MAX_DEPTH = 12
N_NODES = 2 ** MAX_DEPTH - 1     # 4095
N_LEAVES = 2 ** MAX_DEPTH        # 4096
NCORES = 8
REC_W = 62                       # 31 (feat, thr) pairs: levels 7..11 subtree

F32 = mybir.dt.float32
F16 = mybir.dt.float16
I32 = mybir.dt.int32
Alu = mybir.AluOpType
CH = 512


def _build_program(G: int, NG: int, repeat: int = 1):
    S = P * G * NG
    GS = P * G
    nc = bacc.Bacc("TRN2", target_bir_lowering=False, debug=False)

    xs = nc.dram_tensor("xs", [S, INPUT_DIM], F16, kind="ExternalInput")
    xt = nc.dram_tensor("xt", [INPUT_DIM, S], F16, kind="ExternalInput")
    lp = nc.dram_tensor("lp", [N_LEAVES, N_CLASSES], F32, kind="ExternalInput")
    w0 = nc.dram_tensor("w0", [P, P], F16, kind="ExternalInput")
    w1 = nc.dram_tensor("w1", [P, P], F16, kind="ExternalInput")
    thr06 = nc.dram_tensor("thr06", [P, 1], F32, kind="ExternalInput")
    iota = nc.dram_tensor("iota", [P, INPUT_DIM], F16, kind="ExternalInput")
    rectab = nc.dram_tensor("rectab", [P, REC_W], F32, kind="ExternalInput")
    out = nc.dram_tensor("out", [S, N_CLASSES], F16, kind="ExternalOutput")
    smxh = nc.dram_tensor("smxh", [N_LEAVES, N_CLASSES], F16, kind="Internal")

    xs_v = xs[:, :].rearrange("(g t p) f -> p g t f", p=P, t=G)
    xt_v = xt[:, :].rearrange("(k p) s -> p k s", p=P)
    og_all = out[:, :].rearrange("(g t p) c -> p g t c", p=P, t=G)
    lp_r = lp[:, :].rearrange("(p c) k -> p c k", p=P)
    smx_w = smxh[:, :].rearrange("(p c) k -> p c k", p=P)

    with tile.TileContext(nc) as tc:
        with tc.tile_pool(name="cns", bufs=1) as cpool, \
             tc.tile_pool(name="xs", bufs=2) as xspool, \
             tc.tile_pool(name="xt", bufs=2) as xtpool, \
             tc.tile_pool(name="accp", bufs=3, space="PSUM") as apool, \
             tc.tile_pool(name="btp", bufs=3, space="PSUM") as btpool, \
             tc.tile_pool(name="bitsc", bufs=2) as bcpool, \
             tc.tile_pool(name="bitss", bufs=2) as bspool, \
             tc.tile_pool(name="mask", bufs=2) as mpool, \
             tc.tile_pool(name="prod", bufs=2) as ppool, \
             tc.tile_pool(name="sml", bufs=3) as spool, \
             tc.tile_pool(name="rec", bufs=2) as rpool, \
             tc.tile_pool(name="orow", bufs=2) as opool:

            t_w0 = cpool.tile([P, P], F16, tag="w0")
            nc.sync.dma_start(t_w0[:], w0[:, :])
            t_w1 = cpool.tile([P, P], F16, tag="w1")
            nc.sync.dma_start(t_w1[:], w1[:, :])
            t_thr = cpool.tile([P, 1], F32, tag="thr06")
            nc.sync.dma_start(t_thr[:], thr06[:, :])
            t_iota = cpool.tile([P, 1, INPUT_DIM], F16, tag="iota")
            nc.sync.dma_start(t_iota[:],
                              iota[:, :].rearrange("p (o f) -> p o f", o=1))
            t_id = cpool.tile([P, P], F16, tag="ident")
            make_identity(nc, t_id[:])

            # ---- Part 1: softmax table (fp16) ----
            with tc.tile_pool(name="p1", bufs=1) as p1pool:
                t_lp = p1pool.tile([P, 32, N_CLASSES], F32)
                nc.sync.dma_start(t_lp[:], lp_r[:, :, :])
                t_exp = p1pool.tile([P, 32, N_CLASSES], F32)
                nc.scalar.activation(out=t_exp[:], in_=t_lp[:],
                                     func=mybir.ActivationFunctionType.Exp)
                t_sum = p1pool.tile([P, 32, 1], F32)
                nc.vector.tensor_reduce(t_sum[:], t_exp[:],
                                        mybir.AxisListType.X, Alu.add)
                t_rcp = p1pool.tile([P, 32, 1], F32)
                nc.vector.reciprocal(t_rcp[:], t_sum[:])
                t_smh = p1pool.tile([P, 32, N_CLASSES], F16)
                nc.vector.tensor_tensor(
                    out=t_smh[:], in0=t_exp[:],
                    in1=t_rcp[:, :, :].to_broadcast([P, 32, N_CLASSES]),
                    op=Alu.mult)
                nc.sync.dma_start(smx_w[:, :, :], t_smh[:])

            # ---- Part 2: per-group traversal ----
            rep_ctx = tc.For_i(0, repeat, 1) if repeat > 1 else None
            if rep_ctx is not None:
                rep_ctx.__enter__()
            for g in range(NG):
                t_xs = xspool.tile([P, G, INPUT_DIM], F16, tag="xs")
                nc.sync.dma_start(t_xs[:], xs_v[:, g])
                t_xt = xtpool.tile([P, 2, GS], F16, tag="xt")
                nc.sync.dma_start(t_xt[:], xt_v[:, :, g * GS:(g + 1) * GS])

                # levels 0-6 bits (PE matmul + transpose)
                t_bs = bspool.tile([P, G, P], F16, tag="bitss")
                nch = (GS + CH - 1) // CH
                for c in range(nch):
                    lo = c * CH
                    L = min(CH, GS - lo)
                    acc = apool.tile([P, CH], F32, tag="acc")
                    nc.tensor.matmul(acc[:, :L], t_w0[:], t_xt[:, 0, lo:lo + L],
                                     start=True, stop=False)
                    nc.tensor.matmul(acc[:, :L], t_w1[:], t_xt[:, 1, lo:lo + L],
                                     start=False, stop=True)
                    t_bc = bcpool.tile([P, CH], F16, tag="bitsc")
                    nc.vector.tensor_tensor(
                        out=t_bc[:, :L], in0=acc[:, :L],
                        in1=t_thr[:, :].to_broadcast([P, L]), op=Alu.is_gt)
                    for j in range(L // P):
                        bt = btpool.tile([P, P], F16, tag="bt")
                        nc.tensor.transpose(bt[:], t_bc[:, j * P:(j + 1) * P],
                                            t_id[:])
                        nc.scalar.activation(
                            out=t_bs[:, (lo // P) + j, :], in_=bt[:],
                            func=mybir.ActivationFunctionType.Copy)

                # walk levels 0-6 over the bit table
                node = spool.tile([P, G, 1], F32, tag="node")
                nc.vector.tensor_copy(out=node[:], in_=t_bs[:, :, 0:1])
                for d in range(1, 7):
                    W = 2 ** d
                    base = W - 1
                    t_nm = mpool.tile([P, G, 64], F16, tag="nmask")
                    nm = t_nm[:, :, :W]
                    nc.vector.tensor_tensor(
                        out=nm, in0=t_iota[:, :, :W].to_broadcast([P, G, W]),
                        in1=node[:, :, :].to_broadcast([P, G, W]),
                        op=Alu.is_equal)
                    t_pr = ppool.tile([P, G, 64], F16, tag="nprod")
                    pr = t_pr[:, :, :W]
                    nc.vector.tensor_tensor(
                        out=pr, in0=nm, in1=t_bs[:, :, base:base + W],
                        op=Alu.mult)
                    bit = spool.tile([P, G, 1], F32, tag="bit")
                    nc.vector.tensor_reduce(bit[:], pr, mybir.AxisListType.X,
                                            Alu.add)
                    nn = spool.tile([P, G, 1], F32, tag="node")
                    nc.vector.scalar_tensor_tensor(
                        out=nn[:], in0=node[:], scalar=2.0, in1=bit[:],
                        op0=Alu.mult, op1=Alu.add)
                    node = nn
                node7 = node

                # subtree record fetch (baseline-proven per-column indirect)
                reci = spool.tile([P, G], I32, tag="reci")
                nc.vector.tensor_copy(out=reci[:], in_=node7[:])
                t_rec = rpool.tile([P, G, REC_W], F32, tag="rec")
                for t in range(G):
                    nc.gpsimd.indirect_dma_start(
                        out=t_rec[:, t, :], out_offset=None,
                        in_=rectab[:, :],
                        in_offset=bass.IndirectOffsetOnAxis(
                            ap=reci[:, t:t + 1], axis=0))

                # levels 7-11: baseline-style 256-wide selects on fp16 x
                lnode = None
                for d in range(7, MAX_DEPTH):
                    j = d - 7
                    W = 2 ** j
                    if d == 7:
                        ft = t_rec[:, :, 0:2]
                    else:
                        base2 = 2 * (W - 1)
                        t_lm = mpool.tile([P, G, 16], F16, tag="lmask")
                        lm = t_lm[:, :, :W]
                        nc.vector.tensor_tensor(
                            out=lm,
                            in0=t_iota[:, :, :W].to_broadcast([P, G, W]),
                            in1=lnode[:, :, :].to_broadcast([P, G, W]),
                            op=Alu.is_equal)
                        rv = t_rec[:, :, base2:base2 + 2 * W].rearrange(
                            "p g (l c) -> p g c l", c=2)
                        t_p2 = ppool.tile([P, G, 2, 16], F32, tag="lprod")
                        p2 = t_p2[:, :, :, :W]
                        nc.vector.tensor_tensor(
                            out=p2,
                            in0=t_lm[:, :, :W].rearrange(
                                "p g (o w) -> p g o w", o=1).to_broadcast(
                                    [P, G, 2, W]),
                            in1=rv, op=Alu.mult)
                        ftt = spool.tile([P, G, 2], F32, tag="ft")
                        nc.vector.tensor_reduce(ftt[:], p2,
                                                mybir.AxisListType.X, Alu.add)
                        ft = ftt[:, :, :]

                    ftb = spool.tile([P, G, 1], F16, tag="ftb")
                    nc.vector.tensor_copy(out=ftb[:], in_=ft[:, :, 0:1])
                    t_xm = mpool.tile([P, G, INPUT_DIM], F16, tag="xmask")
                    nc.vector.tensor_tensor(
                        out=t_xm[:],
                        in0=t_iota[:, :, :].to_broadcast([P, G, INPUT_DIM]),
                        in1=ftb[:, :, :].to_broadcast([P, G, INPUT_DIM]),
                        op=Alu.is_equal)
                    t_xp = ppool.tile([P, G, INPUT_DIM], F16, tag="xprod")
                    nc.vector.tensor_tensor(out=t_xp[:], in0=t_xm[:],
                                            in1=t_xs[:], op=Alu.mult)
                    val = spool.tile([P, G, 1], F32, tag="val")
                    nc.vector.tensor_reduce(val[:], t_xp[:],
                                            mybir.AxisListType.X, Alu.add)
                    bit = spool.tile([P, G, 1], F32, tag="lbit")
                    nc.vector.tensor_tensor(out=bit[:], in0=val[:],
                                            in1=ft[:, :, 1:2], op=Alu.is_gt)
                    if d == 7:
                        lnode = bit
                    else:
                        ln = spool.tile([P, G, 1], F32, tag="lnode")
                        nc.vector.scalar_tensor_tensor(
                            out=ln[:], in0=lnode[:], scalar=2.0, in1=bit[:],
                            op0=Alu.mult, op1=Alu.add)
                        lnode = ln

                # leaf = node7*32 + lnode; gather softmax rows (fp16)
                leaf = spool.tile([P, G, 1], F32, tag="leaf")
                nc.vector.scalar_tensor_tensor(
                    out=leaf[:], in0=node7[:], scalar=32.0, in1=lnode[:],
                    op0=Alu.mult, op1=Alu.add)
                leafi = spool.tile([P, G], I32, tag="leafi")
                nc.vector.tensor_copy(out=leafi[:], in_=leaf[:])
                t_oh = opool.tile([P, G, N_CLASSES], F16, tag="orowh")
                for t in range(G):
                    nc.gpsimd.indirect_dma_start(
                        out=t_oh[:, t, :], out_offset=None, in_=smxh[:, :],
                        in_offset=bass.IndirectOffsetOnAxis(
                            ap=leafi[:, t:t + 1], axis=0))
                nc.sync.dma_start(og_all[:, g], t_oh[:])

            if rep_ctx is not None:
                rep_ctx.__exit__(None, None, None)

    nc.compile()
    return nc


def _host_tables(split_features, split_thresholds):
    feat = np.clip(np.floor(split_features), 0, INPUT_DIM - 1).astype(np.int64)
    thr = split_thresholds.astype(np.float32)
    featf = feat.astype(np.float32)

    w0 = np.zeros((P, P), np.float16)
    w1 = np.zeros((P, P), np.float16)
    for n in range(127):
        f = int(feat[n])
        if f < 128:
            w0[f, n] = 1.0
        else:
            w1[f - 128, n] = 1.0
    thr06 = np.full((P, 1), 3e38, np.float32)
    thr06[:127, 0] = thr[:127]

    iota = np.broadcast_to(np.arange(INPUT_DIM, dtype=np.float16),
                           (P, INPUT_DIM)).copy()

    rec = np.empty((P, REC_W), np.float32)
    for l7 in range(P):
        for j in range(5):
            W = 2 ** j
            lvl_base = 2 ** (7 + j) - 1
            for l in range(W):
                n = lvl_base + l7 * W + l
                off = 2 * (W - 1 + l)
                rec[l7, off] = featf[n]
                rec[l7, off + 1] = thr[n]
    return w0, w1, thr06, iota, rec


_PROG_CACHE = {}


def kernel(x, split_features, split_thresholds, leaf_probabilities):
    x = np.asarray(x, dtype=np.float32)
    split_features = np.asarray(split_features, dtype=np.float32)
    split_thresholds = np.asarray(split_thresholds, dtype=np.float32)
    leaf_probabilities = np.asarray(leaf_probabilities, dtype=np.float32)

    B = x.shape[0]
    G = 24
    per_core_min = (B + NCORES - 1) // NCORES
    tiles_pc = (per_core_min + P - 1) // P
    NG = (tiles_pc + G - 1) // G
    S = P * G * NG

    w0, w1, thr06, iota, rec = _host_tables(split_features, split_thresholds)

    key = (G, NG)
    nc = _PROG_CACHE.get(key)
    if nc is None:
        nc = _build_program(G, NG)
        _PROG_CACHE[key] = nc

    in_maps = []
    for c in range(NCORES):
        lo = c * S
        hi = min(lo + S, B)
        shard = np.empty((S, INPUT_DIM), np.float32)
        if hi > lo:
            shard[:hi - lo] = x[lo:hi]
            if hi - lo < S:
                shard[hi - lo:] = x[0]
        else:
            shard[:] = x[0]
        sh16 = shard.astype(np.float16)
        m = {"xs": sh16, "xt": np.ascontiguousarray(sh16.T),
             "lp": leaf_probabilities,
             "w0": w0, "w1": w1, "thr06": thr06, "iota": iota,
             "rectab": rec}
        in_maps.append(m)

    res = run_bass_kernel_spmd(nc, in_maps, core_ids=list(range(NCORES)))

    outs = []
    for c in range(NCORES):
        lo = c * S
        hi = min(lo + S, B)
        if hi > lo:
            outs.append(res.results[c]["out"][:hi - lo].astype(np.float32))
    return np.concatenate(outs, axis=0)
